# revision 10
# baseline (speedup 1.0000x reference)
"""Trainium2 Bass kernel for nn_MaxMarginLoss (segment_reduce).

Data-parallel over the batch: 32 samples -> 8 NeuronCores x 4 samples.

Everything derivable from step_ids (segment counts, first-appearance
order, the adjacent-pair adjacency A, pair validity) is integer work on
a [B,T] int tensor -- precomputed on the host, like the baseline's mask
prep.  That lets the whole per-sample pipeline fold into the streaming
matmul: with G = (I - A) @ diag(recip/sqrt(D)) (per sample) and
mask_c the per-chunk one-hot step mask,

    diff = (I - A) @ diag(r) @ segsum(|x|) = sum_c (G @ mask_c^T) @ |x_c|
         = sum_c W_c @ |x_c|

so the device just streams x (the 32 MiB/core memory-bound part),
takes |x| in bf16, and accumulates one matmul per chunk-half directly
into a per-sample PSUM `diff` tile -- no segment-sum PSUM, no scale
copy, no reorder matmul, no subtract.  Per sample the tail is an ACT
relu (DVE cannot read two PSUM operands) plus a fused square-accumulate
and two tiny vector ops; the succ/inv pair weights ride in the final
column-sum matmul's lhsT; the host applies labels and the final scalar
division.

W_c in bf16 only perturbs diff by ~2^-9 relative on the *variance*
part of H (the mean component cancels in H_i - H_next), well inside
the 2e-2 tolerance (measured ~1e-5).
"""

import numpy as np

import concourse.bass as bass
from concourse import mybir
from concourse.bass_utils import run_bass_kernel_spmd
from concourse.tile import TileContext
from concourse.vector_clock import ScopedClock

F32 = mybir.dt.float32
BF16 = mybir.dt.bfloat16
U16 = mybir.dt.uint16
OP = mybir.AluOpType
AF = mybir.ActivationFunctionType

B, T, D = 32, 2048, 1024
S = 32          # step ids 1..32; id 0 is padding
ALPHA = 1.0
N_CORES = 8
BL = B // N_CORES           # samples per core
K = 128                     # matmul contraction tile (partitions)
NCHUNK = T // K             # 16 K-chunks per sample
H2 = D // 2

# Per-sample DMA tiling: 16 KiB contiguous DRAM rows per partition
# (XT=4) run the SDMA engines closest to their ~27 GB/s streaming rate
# (8 KiB rows measure ~25.7 B/ns, 16 KiB ~26.2, 32 KiB drops to ~21);
# the four tapered 512 KiB tail tiles spread the end-of-sample abs work
# across both engines and shrink the exposed critical path after the
# final byte lands.
TILES = [(0, 8), (8, 4), (12, 1), (13, 1), (14, 1), (15, 1)]


def chunk_tmap(c: int) -> np.ndarray:
    """t index per (partition, sub) for chunk c, matching the DMA APs."""
    p = np.arange(K)
    if c < 12:
        return (c // 4) * 512 + 4 * p + (c % 4)
    return c * K + p


# The public neuronxcc walrus (setupSyncWait in CoreV2/V3GenImpl) only
# supports a small number of embedded semaphore waits per instruction,
# while Tile's scheduler attaches one wait per required logical proc.
# After scheduling, hoist overflow waits onto same-engine no-ops placed
# immediately before the owning instruction: engine program order makes
# that semantically identical.
_MAX_WAITS_DEFAULT = 1
_MAX_WAITS_BY_OPCODE = {}


class _LeanTailTileContext(TileContext):
    """Tile's default kernel tail is drain -> barrier -> sem-clear ->
    barrier.  After the first all-engine barrier no engine can still be
    waiting on a kernel semaphore, so the clears need no cross-engine
    ordering and the second (~3-4 us) barrier can be dropped; each
    engine's stream still ends after its own clears, so re-execution
    sees zeroed semaphores."""

    def _drain_and_barrier(self, tick_clock, wait_clock):
        drain_inst = self.nc.sync.drain()
        wait_clock.add_sem_waits(
            drain_inst.ins, ScopedClock({None: tick_clock.global_clock})
        )
        self.nc.all_engine_barrier()
        assert self.sems is not None
        popped = self.nc._tile_sem_poison_stack.pop()
        assert popped is self._sem_poison
        self.nc.clear_and_free_semaphores(list(self.sems.allocated().values()))


def _split_sync_waits(nc: bass.Bass):
    for f in nc.m.functions:
        for bb in f.blocks:
            insts = list(bb.instructions)
            need = []  # (ins, overflow_waits)
            for ins in insts:
                si = getattr(ins, "sync_info", None)
                if si is None or not si.on_wait:
                    continue
                cap = _MAX_WAITS_BY_OPCODE.get(ins.opcode, _MAX_WAITS_DEFAULT)
                waits = list(si.on_wait)
                if len(waits) <= cap:
                    continue
                ins.sync_info = mybir.SyncInfo(
                    on_wait=waits[:cap], on_update=list(si.on_update)
                )
                need.append((ins, waits[cap:], cap))
            if not need:
                continue
            nop_for: dict[str, list] = {}
            for ins, overflow, cap in need:
                eng = nc.engines[ins.engine]
                nops = []
                for i in range(0, len(overflow), cap):
                    nop = eng.nop(hint="waitsplit", nofuse=True)
                    nop.ins.sync_info = mybir.SyncInfo(
                        on_wait=overflow[i:i + cap], on_update=[]
                    )
                    nops.append(nop.ins)
                nop_for[ins.name] = nops
            created = {n.name for nops in nop_for.values() for n in nops}
            # nop() appended the new instructions to the current bb; pull
            # them out of every block and splice before their owners.
            for bb2 in f.blocks:
                cur = [i for i in bb2.instructions if i.name not in created]
                out = []
                for ins in cur:
                    out.extend(nop_for.get(ins.name, ()))
                    out.append(ins)
                bb2.instructions = out


def _strip_constructor_tail(nc: bass.Bass, names: set[str]):
    """Drop the Bass-constructor const-AP memsets (this kernel never
    reads the const APs) and the constructor's all-engine barrier (the
    body's cross-engine ordering is fully semaphore-driven; engine
    streams are self-ordered against their own preamble).  Saves ~1 us
    of serial startup before the first DMA issue."""
    drop_ops = {"Memset", "Drain", "EventSemaphore"}
    for f in nc.m.functions:
        for bb in f.blocks:
            bb.instructions = [
                i for i in bb.instructions
                if not (i.name in names and i.opcode in drop_ops)
            ]


def build_program() -> bass.Bass:
    nc = bass.Bass()
    ctor_names = {
        i.name for f in nc.m.functions for bb in f.blocks
        for i in bb.instructions
    }

    x = nc.declare_dram_parameter("x", [BL, T, D], F32, isOutput=False)
    # W_c lhsT blocks for every (sample, chunk): bf16 bit patterns.
    wt16 = nc.declare_dram_parameter(
        "wt16", [K, BL * NCHUNK * S], U16, isOutput=False
    )
    # cols 0-3: succ * block-diag ones, cols 4-7: inv * block-diag ones
    # (lhsT of the final column-sum matmul).
    cf = nc.declare_dram_parameter("cf", [K, 8], F32, isOutput=False)
    out3 = nc.declare_dram_parameter("out3", [8, 3], F32, isOutput=True)

    with _LeanTailTileContext(nc) as tc:
        with (
            tc.tile_pool(name="const", bufs=1) as cpool,
            tc.tile_pool(name="persist", bufs=1) as pp,
            tc.tile_pool(name="xin8", bufs=2) as xin8,
            tc.tile_pool(name="xin4", bufs=2) as xin4,
            tc.tile_pool(name="xin1", bufs=4) as xin1,
            tc.tile_pool(name="xa8", bufs=2) as xa8,
            tc.tile_pool(name="xa4", bufs=2) as xa4,
            tc.tile_pool(name="xa1", bufs=4) as xa1,
            tc.tile_pool(name="ps", bufs=1, space="PSUM") as psp,
        ):
            sb_wt = cpool.tile([K, BL * NCHUNK * S], U16)
            sb_cf = cpool.tile([K, 8], F32)

            # diff accumulates across all 16 chunks of each sample in
            # rows [32b, 32b+32); samples use disjoint partition groups
            # so one 2-bank tile serves all four.
            diff = psp.tile([K, D], F32)
            vp = psp.tile([8, 8], F32)
            relu_sb = pp.tile([K, D], BF16)
            sq = pp.tile([K, D], BF16)
            er3 = pp.tile([K, 3], F32)   # e_half0, e_half1, relu(1-E)
            ae = pp.tile([K, 1], F32)

            # Emitted one sample late so the ACT/DVE ops land in their
            # queues after their dependencies are met (emitted eagerly
            # they head-of-line-block the abs stream behind sample b's
            # last matmul and stall the DMAs).
            def sample_tail(b):
                bs = slice(b * S, (b + 1) * S)
                # E_i = sum_d relu(diff)^2 (the 1/D mean and recip are
                # folded into W).  DVE can't read both multiplicands
                # from PSUM, so ACT takes the relu (PSUM -> SBUF bf16,
                # per d-half so the last sample pipelines against the
                # final matmuls) and DVE squares at the 16-bit rate with
                # the free-dim sum fused in (max-with-0 is an identity
                # on relu'd values).
                for h in range(2):
                    hs = slice(h * H2, (h + 1) * H2)
                    nc.scalar.activation(
                        relu_sb[bs, hs], diff[bs, hs], AF.Relu
                    )
                for h in range(2):
                    hs = slice(h * H2, (h + 1) * H2)
                    nc.vector.scalar_tensor_tensor(
                        sq[bs, hs], relu_sb[bs, hs], 0.0, relu_sb[bs, hs],
                        op0=OP.max, op1=OP.mult,
                        accum_out=er3[bs, h:h + 1],
                    )
                # relu(ALPHA - e0 - e1) in two ops:
                # ae = (e0 * -1) - e1, then max(ae + ALPHA, 0).
                nc.vector.scalar_tensor_tensor(
                    ae[bs, :], er3[bs, 0:1], -1.0, er3[bs, 1:2],
                    op0=OP.mult, op1=OP.subtract,
                )
                nc.vector.tensor_scalar(
                    er3[bs, 2:3], ae[bs, :], ALPHA, 0.0, OP.add, OP.max
                )

            ti = 0
            for b in range(BL):
                for tix, (c0, xt) in enumerate(TILES):
                    if b > 0 and tix == 1:
                        sample_tail(b - 1)
                    # All x DMAs on the sync ring: one HWDGE ring's
                    # descriptor feed saturates all 16 SDMA engines.
                    # The xt=8 tile is ONE dma_start covering chunks
                    # 0-7 as two 16 KiB segments per partition (same
                    # row size as the XT4 tiles -- 32 KiB rows measure
                    # ~20% slower); one sem instead of two.
                    if xt == 8:
                        xtile = xin8.tile([K, 2, 4, D], F32)
                        src = x[b, 0:1024, :].rearrange(
                            "(g p s) d -> p g s d", g=2, p=K
                        )
                        nc.sync.dma_start(out=xtile[:], in_=src)
                        xa = xa8.tile([K, 2, 4, D], BF16)
                    else:
                        xpool, apool = {
                            4: (xin4, xa4), 1: (xin1, xa1)
                        }[xt]
                        xtile = xpool.tile([K, xt, D], F32)
                        src = x[b, c0 * K:(c0 + xt) * K, :].rearrange(
                            "(p s) d -> p s d", p=K
                        )
                        nc.sync.dma_start(out=xtile[:], in_=src)
                        xa = apool.tile([K, xt, D], BF16)
                    if b == 0 and tix == 0:
                        # Constants follow the first x tile on the same
                        # ring: the stream's first byte lands ~1.3 us
                        # earlier than if W went first, and W still
                        # arrives before the first matmul wants it.
                        nc.sync.dma_start(out=sb_wt[:], in_=wt16[:])
                        nc.sync.dma_start(out=sb_cf[:], in_=cf[:])

                    # |x| rounded to bf16: PE runs bf16 at 1 cycle/row
                    # vs fp32's 4; the 2^-9 rounding washes out in the
                    # loss.  Both engines split every tile (ACT: Abs
                    # activation; DVE: cast+sign-clear) so neither
                    # backs up near the stream's end.
                    last_tile = b == BL - 1 and tix == len(TILES) - 1
                    def dve_abs(dst, srcap):
                        # DVE abs: cast f32->bf16 (RNE, so |bf16(x)| ==
                        # bf16(|x|)) then clear the sign bit in the
                        # 16-bit 4x mode.
                        nc.vector.tensor_copy(dst, srcap)
                        nc.vector.tensor_scalar(
                            dst.bitcast(U16), dst.bitcast(U16),
                            0x7FFF, None, OP.bitwise_and,
                        )

                    if xt == 8:
                        nc.scalar.activation(
                            xa[:, 0, :, :], xtile[:, 0, :, :], AF.Abs
                        )
                        dve_abs(xa[:, 1, :, :], xtile[:, 1, :, :])
                    elif xt == 4:
                        nc.scalar.activation(
                            xa[:, 0:2, :], xtile[:, 0:2, :], AF.Abs
                        )
                        dve_abs(xa[:, 2:4, :], xtile[:, 2:4, :])
                    elif last_tile:
                        # Split the final tile's abs across both engines
                        # so each matmul half starts as soon as possible.
                        nc.scalar.activation(
                            xa[:, 0, 0:H2], xtile[:, 0, 0:H2], AF.Abs
                        )
                        dve_abs(xa[:, 0, H2:D], xtile[:, 0, H2:D])
                    elif c0 == 12:
                        nc.scalar.activation(xa[:], xtile[:], AF.Abs)
                    else:
                        dve_abs(xa[:], xtile[:])
                    ti += 1

                    for sub in range(xt):
                        c = c0 + sub
                        wcol = (b * NCHUNK + c) * S
                        if xt == 8:
                            rsl = xa[:, sub // 4, sub % 4, :]
                        else:
                            rsl = xa[:, sub, :]
                        for h in range(2):
                            nc.tensor.matmul(
                                diff[b * S:(b + 1) * S, h * H2:(h + 1) * H2],
                                lhsT=sb_wt[:, wcol:wcol + S].bitcast(BF16),
                                rhs=rsl[:, h * H2:(h + 1) * H2],
                                start=(c == 0), stop=(c == NCHUNK - 1),
                                tile_position=(0, b * S),
                            )

            sample_tail(BL - 1)

            # s1/s2 column sums with succ/inv folded into the lhsT:
            # out[b, 0] + out[b, 1] = sum_i succ_i E_i (halves), and
            # out[4+b, 2] = sum_i inv_i relu(1-E_i).
            nc.tensor.matmul(
                vp[:, 0:3], lhsT=sb_cf[:], rhs=er3[:],
                start=True, stop=True,
            )
            out_sb = pp.tile([8, 3], F32)
            nc.vector.tensor_copy(out_sb[:], vp[:, 0:3])
            nc.sync.dma_start(out=out3[:], in_=out_sb[:])

    _split_sync_waits(nc)
    _strip_constructor_tail(nc, ctor_names)
    return nc


_PROGRAM: bass.Bass | None = None


def get_program() -> bass.Bass:
    global _PROGRAM
    if _PROGRAM is None:
        _PROGRAM = build_program()
    return _PROGRAM


def _f32_to_bf16_bits(a: np.ndarray) -> np.ndarray:
    """Round-to-nearest-even f32 -> bf16 bit patterns (uint16)."""
    u = np.ascontiguousarray(a, dtype=np.float32).view(np.uint32)
    rnd = ((u >> 16) & 1) + np.uint32(0x7FFF)
    return ((u + rnd) >> 16).astype(np.uint16)


def host_prep(step_ids: np.ndarray):
    """Per-sample index math (all integer work on step_ids) plus the
    per-chunk W lhsT blocks.  Returns (per-core in_map extras, per-sample
    scalars for the final host combine)."""
    step_ids = np.asarray(step_ids)
    rsqrt_d = 1.0 / np.sqrt(np.float64(D))

    wt_all = np.empty((B, NCHUNK, K, S), dtype=np.float32)
    succ_all = np.empty((B, S), dtype=np.float32)
    inv_all = np.empty((B, S), dtype=np.float32)
    npairs_all = np.empty(B, dtype=np.int64)
    n_all = np.empty(B, dtype=np.int64)
    ninv_all = np.empty(B, dtype=np.int64)

    steps = np.arange(1, S + 1)
    tmaps = np.stack([chunk_tmap(c) for c in range(NCHUNK)])  # [NCHUNK, K]

    for gb in range(B):
        ids = step_ids[gb]                                   # [T]
        mask = ids[:, None] == steps[None, :]                # [T, S]
        counts = mask.sum(axis=0)
        recip = 1.0 / np.maximum(counts, 1.0)
        pos = np.where(mask, np.arange(T)[:, None], T).min(axis=0)
        perm = np.argsort(pos, kind="stable")
        ordered_steps = steps[perm]
        present_slot = pos[perm] < T
        n = int(present_slot.sum())

        # row i = step id i+1; rank = slot index in appearance order
        rank = np.empty(S, dtype=np.int64)
        rank[perm] = np.arange(S)

        succ = np.zeros(S, dtype=np.float32)
        inv = np.zeros(S, dtype=np.float32)
        G = np.zeros((S, S), dtype=np.float64)
        for i in range(S):
            k = rank[i]
            if k + 1 < S and present_slot[k] and present_slot[k + 1]:
                nxt = perm[k + 1]
                succ[i] = 1.0
                if ordered_steps[k] > ordered_steps[k + 1]:
                    inv[i] = 1.0
                G[i, i] = recip[i] * rsqrt_d
                G[i, nxt] -= recip[nxt] * rsqrt_d

        npairs_all[gb] = int(succ.sum())
        n_all[gb] = n
        ninv_all[gb] = int(inv.sum())
        succ_all[gb] = succ
        inv_all[gb] = inv

        # W_c^T[p, i] = G[i, s_p] for the step s_p at t = tmap(c, p)
        ids_c = ids[tmaps]                                   # [NCHUNK, K]
        valid = ids_c >= 1
        gidx = np.clip(ids_c - 1, 0, S - 1)
        wt_all[gb] = np.where(
            valid[:, :, None], G.T[gidx, :], 0.0
        ).astype(np.float32)

    return wt_all, succ_all, inv_all, npairs_all, n_all, ninv_all


def make_in_maps(inputs: np.ndarray, step_ids: np.ndarray):
    inputs = np.ascontiguousarray(np.asarray(inputs, dtype=np.float32))
    wt_all, succ_all, inv_all, npairs, n, ninv = host_prep(step_ids)

    bones = ((np.arange(K)[:, None] // S) == np.arange(BL)[None, :]).astype(
        np.float32
    )

    in_maps = []
    for core in range(N_CORES):
        b0 = core * BL
        # [K, BL*NCHUNK*S] with the (sample, chunk) blocks contiguous
        wt = wt_all[b0:b0 + BL].transpose(2, 0, 1, 3).reshape(K, -1)
        cf = np.empty((K, 8), dtype=np.float32)
        cf[:, 0:4] = succ_all[b0:b0 + BL].reshape(K)[:, None] * bones
        cf[:, 4:8] = inv_all[b0:b0 + BL].reshape(K)[:, None] * bones
        in_maps.append({
            "x": inputs[b0:b0 + BL],
            "wt16": _f32_to_bf16_bits(wt),
            "cf": cf,
        })
    return in_maps, (npairs, n, ninv)


def finish_host(out3_per_core, stats, binary_labels) -> np.float32:
    npairs, n, ninv = stats
    s1 = np.concatenate(
        [np.asarray(o, np.float64)[0:4, 0] + np.asarray(o, np.float64)[0:4, 1]
         for o in out3_per_core]
    )
    s2 = np.concatenate(
        [np.asarray(o, np.float64)[4:8, 2] for o in out3_per_core]
    )
    labels = np.asarray(binary_labels)
    loss_pos = s1 / np.maximum(npairs, 1.0)
    loss_neg = s2 / np.maximum(ninv, 1.0)
    pos_count = (labels == 1) & (n >= 2)
    neg_count = (labels == 0) & (ninv > 0)
    total = (loss_pos * pos_count).sum() + (loss_neg * neg_count).sum()
    num = pos_count.sum() + neg_count.sum()
    return np.float32(total / (num + 1e-9))


def kernel(inputs, step_ids, binary_labels, _trace=False):
    nc = get_program()
    in_maps, stats = make_in_maps(inputs, step_ids)
    res = run_bass_kernel_spmd(
        nc, in_maps, core_ids=list(range(N_CORES)), trace=_trace
    )
    out = finish_host([r["out3"] for r in res.results], stats, binary_labels)
    if _trace:
        return out, res
    return out
